# revision 2
# baseline (speedup 1.0000x reference)
"""MultiHeadCoAttention Trainium2 Bass kernel, 8-way head-parallel SPMD.

Contract: kernel(**inputs) takes the full (unsharded) inputs of the reference
nn.Module and returns the full output tuple (out_q, out_c).

Sharding strategy (hardcoded for B=2, Lq=Lc=2048, D=1024, H=16, dk=64, 8 cores):
  - core k owns heads {2k, 2k+1} for both batches (head-parallel attention);
  - all four input projections, scores, the two softmaxes and both attention
    applies for those heads run fully on-core with no communication;
  - softmax is computed max-free (scores are O(5), exp is exact in fp32) with
    the row/col sums obtained for free as an extra ones-column in the value
    matmuls, so only one exp pass per score orientation is needed;
  - an on-device AllToAll redistributes the per-head results from
    [d-slice, all tokens] to [all d, token-slice], after which each core
    computes the two output linears for its 512-token slice only;
  - host side only slices/casts weights and concatenates the 8 token-slices.
Compute dtype is fp16 (PE runs fp16 at full rate vs 4x slower fp32) with fp32
PSUM accumulation everywhere.
"""

import numpy as np

B, LQ, LC, D, H, DK = 2, 2048, 2048, 1024, 16, 64
N_CORES = 8
HPC = H // N_CORES          # heads per core = 2
DSL = HPC * DK              # d-slice width per core = 128
LTOT = B * LQ               # 4096 flattened token rows
LSL = LTOT // N_CORES       # 512 token rows per core
NKT = D // 128              # 8 k-tiles over the model dim
NLT = LQ // 128             # 16 l-tiles per batch
SCALE = 1.0 / float(np.sqrt(DK))

_CACHE = {}


def _build_program():
    import concourse.bacc as bacc
    import concourse.mybir as mybir
    from concourse import tile

    f32 = mybir.dt.float32
    f16 = mybir.dt.float16
    Exp = mybir.ActivationFunctionType.Exp
    add = mybir.AluOpType.add
    mult = mybir.AluOpType.mult

    nc = bacc.Bacc("TRN2", target_bir_lowering=False, debug=False,
                   num_devices=N_CORES)

    # ---- I/O ----
    query = nc.dram_tensor("query", [B, LQ, D], f32, kind="ExternalInput")
    context = nc.dram_tensor("context", [B, LC, D], f32, kind="ExternalInput")
    w0t = nc.dram_tensor("w0t", [D, DSL], f16, kind="ExternalInput")
    w1t = nc.dram_tensor("w1t", [D, DSL], f16, kind="ExternalInput")
    w2t = nc.dram_tensor("w2t", [D, DSL], f16, kind="ExternalInput")
    w3t = nc.dram_tensor("w3t", [D, DSL], f16, kind="ExternalInput")
    w4t = nc.dram_tensor("w4t", [D, D], f16, kind="ExternalInput")
    w5t = nc.dram_tensor("w5t", [D, D], f16, kind="ExternalInput")
    b0s = nc.dram_tensor("b0s", [DSL, 1], f32, kind="ExternalInput")
    b1s = nc.dram_tensor("b1s", [DSL, 1], f32, kind="ExternalInput")
    b2r = nc.dram_tensor("b2r", [128, DSL], f32, kind="ExternalInput")
    b3r = nc.dram_tensor("b3r", [128, DSL], f32, kind="ExternalInput")
    b4r = nc.dram_tensor("b4r", [128, D], f32, kind="ExternalInput")
    b5r = nc.dram_tensor("b5r", [128, D], f32, kind="ExternalInput")
    ident = nc.dram_tensor("ident", [128, 128], f16, kind="ExternalInput")
    out0c = nc.dram_tensor("out0c", [LSL, D], f32, kind="ExternalOutput")
    out1c = nc.dram_tensor("out1c", [LSL, D], f32, kind="ExternalOutput")

    with tile.TileContext(nc) as tc:
        with tc.tile_pool(name="dram", bufs=1, space="DRAM") as dram, \
             tc.tile_pool(name="const", bufs=1) as constp, \
             tc.tile_pool(name="psA", bufs=3, space="PSUM") as psA, \
             tc.tile_pool(name="psB", bufs=2, space="PSUM") as psB:

            # fp16 staging of the two activations (cast on SWDGE)
            stage_q = dram.tile([B, LQ, D], f16)
            stage_c = dram.tile([B, LC, D], f16)
            a2a_in = dram.tile([N_CORES, 2, DSL, LSL], f16)
            a2a_out = dram.tile([N_CORES, 2, DSL, LSL], f16)

            for b in range(B):
                nc.gpsimd.dma_start(stage_q[b], query.ap()[b])
                nc.gpsimd.dma_start(stage_c[b], context.ap()[b])

            # constants / weights that live for the whole kernel
            idt = constp.tile([128, 128], f16, name="idt")
            nc.sync.dma_start(idt[:], ident.ap())
            bias_qp = constp.tile([DSL, 1], f32, name="bias_qp")
            nc.sync.dma_start(bias_qp[:], b0s.ap())
            bias_cp = constp.tile([DSL, 1], f32, name="bias_cp")
            nc.sync.dma_start(bias_cp[:], b1s.ap())
            bias_qv = constp.tile([128, DSL], f32, name="bias_qv")
            nc.sync.dma_start(bias_qv[:], b2r.ap())
            bias_cv = constp.tile([128, DSL], f32, name="bias_cv")
            nc.sync.dma_start(bias_cv[:], b3r.ap())
            wq = [constp.tile([128, DSL], f16, name=f"wq{k}") for k in range(NKT)]
            wc = [constp.tile([128, DSL], f16, name=f"wc{k}") for k in range(NKT)]
            wqv = [constp.tile([128, DSL], f16, name=f"wqv{k}") for k in range(NKT)]
            wcv = [constp.tile([128, DSL], f16, name=f"wcv{k}") for k in range(NKT)]
            for k in range(NKT):
                sl = slice(128 * k, 128 * (k + 1))
                nc.sync.dma_start(wq[k][:], w0t.ap()[sl])
                nc.sync.dma_start(wc[k][:], w1t.ap()[sl])
                nc.sync.dma_start(wqv[k][:], w2t.ap()[sl])
                nc.sync.dma_start(wcv[k][:], w3t.ap()[sl])

            # ---- phase 1: transpose inputs + all projections ----
            # outputs (live until end of attention):
            #   qTp/cTp [128(2 heads x dk), 2048] per batch  (score projections)
            #   qvv/cvv [128(l), 65] per (batch, ltile, head) with ones col 64
            with tc.tile_pool(name="proj", bufs=1) as projp:
                qTp = [projp.tile([128, LQ], f16, name=f"qTp{b}") for b in range(B)]
                cTp = [projp.tile([128, LC], f16, name=f"cTp{b}") for b in range(B)]
                qvv = [[[projp.tile([128, DK + 1], f16, name=f"qvv{b}_{lt}_{h}")
                         for h in range(HPC)] for lt in range(NLT)] for b in range(B)]
                cvv = [[[projp.tile([128, DK + 1], f16, name=f"cvv{b}_{lt}_{h}")
                         for h in range(HPC)] for lt in range(NLT)] for b in range(B)]

                with tc.tile_pool(name="inT", bufs=2 * NKT) as inp:
                    for b in range(B):
                        # transposed fp16 activations: [128(d), 2048(l)] x 8
                        qT = [inp.tile([128, LQ], f16, tag="qT", name=f"qT{b}_{k}") for k in range(NKT)]
                        cT = [inp.tile([128, LC], f16, tag="cT", name=f"cT{b}_{k}") for k in range(NKT)]
                        for k in range(NKT):
                            dsl = slice(128 * k, 128 * (k + 1))
                            nc.sync.dma_start(qT[k][:], stage_q[b, :, dsl],
                                              transpose=True)
                            nc.sync.dma_start(cT[k][:], stage_c[b, :, dsl],
                                              transpose=True)
                        # score projections: lhsT = wq ktile, rhs = qT ktile
                        for (dst, w_, src, bias) in ((qTp, wq, qT, bias_qp),
                                                     (cTp, wc, cT, bias_cp)):
                            for ch in range(LQ // 512):
                                cs = slice(512 * ch, 512 * (ch + 1))
                                ps = psB.tile([128, 512], f32, tag="pss", name="ps")
                                for k in range(NKT):
                                    nc.tensor.matmul(ps[:], w_[k][:], src[k][:, cs],
                                                     start=(k == 0),
                                                     stop=(k == NKT - 1))
                                nc.vector.tensor_scalar(
                                    out=dst[b][:, cs], in0=ps[:],
                                    scalar1=bias[:, 0:1], scalar2=None, op0=add)
                        # value projections: lhsT = qT ktile l-slab, rhs = w
                        for (dst, w_, src, bias) in ((qvv, wqv, qT, bias_qv),
                                                     (cvv, wcv, cT, bias_cv)):
                            for lt in range(NLT):
                                ls = slice(128 * lt, 128 * (lt + 1))
                                ps = psB.tile([128, DSL], f32, tag="pss", name="ps")
                                for k in range(NKT):
                                    nc.tensor.matmul(ps[:], src[k][:, ls], w_[k][:],
                                                     start=(k == 0),
                                                     stop=(k == NKT - 1))
                                for h in range(HPC):
                                    hs = slice(DK * h, DK * (h + 1))
                                    t = dst[b][lt][h]
                                    nc.vector.tensor_tensor(
                                        out=t[:, 0:DK], in0=ps[:, hs],
                                        in1=bias[:, hs], op=add)
                                    nc.vector.memset(t[:, DK:DK + 1], 1.0)

                # ---- phase 2: attention per (batch, head, side) ----
                # side 0 (U): S^T c-tiles -> exp -> Et ; U' = Et^T-slab @ cvv
                # side 1 (V): S  q-tiles -> exp -> E  ; V' = E-slab @ qvv
                # results (normalized) land in rq/rc [128(l), 128(2 heads)] f16,
                # then PE-transpose into rqt/rct [128(d), 2048(l)] per batch.
                with tc.tile_pool(name="att", bufs=1) as attp, \
                     tc.tile_pool(name="emat", bufs=26) as ematp:
                    rq = [[attp.tile([128, 128], f16, name=f"rq{b}_{m}")
                           for m in range(NLT)] for b in range(B)]
                    rc = [[attp.tile([128, 128], f16, name=f"rc{b}_{m}")
                           for m in range(NLT)] for b in range(B)]
                    rqt = [attp.tile([128, LQ], f16, name=f"rqt{b}") for b in range(B)]
                    rct = [attp.tile([128, LC], f16, name=f"rct{b}") for b in range(B)]

                    for b in range(B):
                        for h in range(HPC):
                            hp = slice(64 * h, 64 * (h + 1))
                            for side in range(2):
                                if side == 0:   # q_p softmax -> q_res
                                    lhsp, rhsp = cTp[b], qTp[b]
                                    vals, rdst = cvv[b], rq[b]
                                else:           # c_p softmax -> c_res
                                    lhsp, rhsp = qTp[b], cTp[b]
                                    vals, rdst = qvv[b], rc[b]
                                # scores + exp, k-tile by k-tile
                                et = []
                                for kt in range(NLT):
                                    ks = slice(128 * kt, 128 * (kt + 1))
                                    e = ematp.tile([128, 2048], f16, tag="et", name="e")
                                    for ch in range(2):
                                        cs = slice(1024 * ch, 1024 * (ch + 1))
                                        sp = psA.tile([128, 1024], f32, tag="sps", name="sp")
                                        nc.tensor.matmul(sp[:, 0:512],
                                                         lhsp[hp, ks],
                                                         rhsp[hp, 1024 * ch:1024 * ch + 512],
                                                         start=True, stop=True)
                                        nc.tensor.matmul(sp[:, 512:1024],
                                                         lhsp[hp, ks],
                                                         rhsp[hp, 1024 * ch + 512:1024 * (ch + 1)],
                                                         start=True, stop=True)
                                        nc.scalar.activation(e[:, cs], sp[:], Exp,
                                                             scale=SCALE)
                                    et.append(e)
                                # apply + normalize per output l-tile
                                for m in range(NLT):
                                    ms = slice(128 * m, 128 * (m + 1))
                                    up = psB.tile([128, DK + 1], f32, tag="pss", name="up")
                                    for kt in range(NLT):
                                        nc.tensor.matmul(up[:], et[kt][:, ms],
                                                         vals[kt][h][:],
                                                         start=(kt == 0),
                                                         stop=(kt == NLT - 1))
                                    rec = attp.tile([128, 1], f32, tag="rec", bufs=4, name="rec")
                                    nc.vector.reciprocal(rec[:], up[:, DK:DK + 1])
                                    nc.vector.tensor_scalar(
                                        out=rdst[m][:, hp], in0=up[:, 0:DK],
                                        scalar1=rec[:, 0:1], scalar2=None, op0=mult)

                        # transpose result shards into [d, l] layout
                        for (r, rt) in ((rq, rqt), (rc, rct)):
                            for m in range(NLT):
                                ms = slice(128 * m, 128 * (m + 1))
                                tp = psB.tile([128, 128], f16, tag="pss", name="tp")
                                nc.tensor.transpose(tp[:], r[b][m][:], idt[:])
                                nc.vector.tensor_copy(rt[b][:, ms], tp[:])

                    # ship shards to the A2A staging buffer
                    for b in range(B):
                        for j in range(4):
                            js = slice(512 * j, 512 * (j + 1))
                            nc.sync.dma_start(a2a_in[4 * b + j, 0], rqt[b][:, js])
                            nc.sync.dma_start(a2a_in[4 * b + j, 1], rct[b][:, js])

            nc.gpsimd.collective_compute(
                "AllToAll", mybir.AluOpType.bypass,
                replica_groups=[list(range(N_CORES))],
                ins=[a2a_in.opt()], outs=[a2a_out.opt()])

            # ---- phase 3: output projections on own 512-token slice ----
            with tc.tile_pool(name="outp", bufs=1) as outp:
                w4 = [outp.tile([128, D], f16, name=f"w4_{k}") for k in range(NKT)]
                w5 = [outp.tile([128, D], f16, name=f"w5_{k}") for k in range(NKT)]
                bias4 = outp.tile([128, D], f32, name="bias4")
                bias5 = outp.tile([128, D], f32, name="bias5")
                nc.sync.dma_start(bias4[:], b4r.ap())
                nc.sync.dma_start(bias5[:], b5r.ap())
                for k in range(NKT):
                    sl = slice(128 * k, 128 * (k + 1))
                    nc.sync.dma_start(w4[k][:], w4t.ap()[sl])
                    nc.sync.dma_start(w5[k][:], w5t.ap()[sl])
                rqf = [outp.tile([128, LSL], f16, name=f"rqf{k}") for k in range(NKT)]
                rcf = [outp.tile([128, LSL], f16, name=f"rcf{k}") for k in range(NKT)]
                for k in range(NKT):
                    nc.sync.dma_start(rqf[k][:], a2a_out[k, 0])
                    nc.sync.dma_start(rcf[k][:], a2a_out[k, 1])

                for (rf, w_, bias, out_) in ((rqf, w4, bias4, out0c),
                                             (rcf, w5, bias5, out1c)):
                    for mt in range(LSL // 128):
                        ms = slice(128 * mt, 128 * (mt + 1))
                        for ch in range(D // 512):
                            cs = slice(512 * ch, 512 * (ch + 1))
                            ps = psB.tile([128, 512], f32, tag="pss", name="ps")
                            for k in range(NKT):
                                nc.tensor.matmul(ps[:], rf[k][:, ms], w_[k][:, cs],
                                                 start=(k == 0), stop=(k == NKT - 1))
                            ev = outp.tile([128, 512], f32, tag="oev", bufs=3, name="ev")
                            nc.vector.tensor_tensor(out=ev[:], in0=ps[:],
                                                    in1=bias[:, cs], op=add)
                            nc.sync.dma_start(out_.ap()[ms, cs], ev[:])

    nc.compile()
    return nc


def _prep_inputs(inputs):
    f16 = np.float16
    f32 = np.float32
    q = np.ascontiguousarray(np.asarray(inputs["query"], dtype=f32))
    c = np.ascontiguousarray(np.asarray(inputs["context"], dtype=f32))
    W = [np.asarray(inputs[f"W{i}"], dtype=f32) for i in range(6)]
    bias = [np.asarray(inputs[f"b{i}"], dtype=f32) for i in range(6)]
    ident = np.eye(128, dtype=f16)
    in_maps = []
    for k in range(N_CORES):
        dsl = slice(DSL * k, DSL * (k + 1))
        m = {
            "query": q,
            "context": c,
            "w0t": np.ascontiguousarray(W[0][dsl].T.astype(f16)),
            "w1t": np.ascontiguousarray(W[1][dsl].T.astype(f16)),
            "w2t": np.ascontiguousarray(W[2][dsl].T.astype(f16)),
            "w3t": np.ascontiguousarray(W[3][dsl].T.astype(f16)),
            "w4t": np.ascontiguousarray(W[4].T.astype(f16)),
            "w5t": np.ascontiguousarray(W[5].T.astype(f16)),
            "b0s": np.ascontiguousarray(bias[0][dsl].reshape(DSL, 1)),
            "b1s": np.ascontiguousarray(bias[1][dsl].reshape(DSL, 1)),
            "b2r": np.ascontiguousarray(np.tile(bias[2][dsl], (128, 1))),
            "b3r": np.ascontiguousarray(np.tile(bias[3][dsl], (128, 1))),
            "b4r": np.ascontiguousarray(np.tile(bias[4], (128, 1))),
            "b5r": np.ascontiguousarray(np.tile(bias[5], (128, 1))),
            "ident": ident,
        }
        in_maps.append(m)
    return in_maps


def _get_program():
    if "nc" not in _CACHE:
        _CACHE["nc"] = _build_program()
    return _CACHE["nc"]


def kernel(**inputs):
    from concourse.bass_utils import run_bass_kernel_spmd

    nc = _get_program()
    in_maps = _prep_inputs(inputs)
    res = run_bass_kernel_spmd(nc, in_maps, list(range(N_CORES)))
    out0 = np.concatenate([res.results[k]["out0c"] for k in range(N_CORES)], axis=0)
    out1 = np.concatenate([res.results[k]["out1c"] for k in range(N_CORES)], axis=0)
    return (out0.reshape(B, LQ, D).astype(np.float32),
            out1.reshape(B, LC, D).astype(np.float32))


# revision 6
# speedup vs baseline: 1.0387x; 1.0387x over previous
"""MultiHeadCoAttention Trainium2 Bass kernel, 8-way head-parallel SPMD.

Contract: kernel(**inputs) takes the full (unsharded) inputs of the reference
nn.Module and returns the full output tuple (out_q, out_c).

Sharding strategy (hardcoded for B=2, Lq=Lc=2048, D=1024, H=16, dk=64, 8 cores):
  - core k owns heads {2k, 2k+1} for both batches (head-parallel attention);
  - all four input projections, scores, the two softmaxes and both attention
    applies for those heads run fully on-core with no communication;
  - softmax is computed max-free (scores are O(5), exp is exact in fp32) with
    the row/col sums obtained for free as an extra ones-column in the value
    matmuls, so only one exp pass per score orientation is needed;
  - the col-softmax orientation exp(S) is either recomputed (scores+exp) or,
    for half the pairs, produced by transposing the row-orientation exp(S^T)
    through a DRAM round-trip on the (otherwise idle) DMA engines — trading
    ScalarE exp time for DMA bandwidth;
  - two on-device AllToAlls redistribute per-head results from
    [d-slice, all tokens] to [all d, token-slice]; the q-side one fires as
    soon as the row-softmax half is done, so its latency and the whole out_q
    output projection hide under the col-softmax compute;
  - host side only slices/casts weights and concatenates the 8 token-slices.
Compute dtype is fp16 (PE runs fp16 at full rate vs 4x slower fp32) with fp32
PSUM accumulation everywhere.
"""

import numpy as np

B, LQ, LC, D, H, DK = 2, 2048, 2048, 1024, 16, 64
N_CORES = 8
HPC = H // N_CORES          # heads per core = 2
DSL = HPC * DK              # d-slice width per core = 128
LTOT = B * LQ               # 4096 flattened token rows
LSL = LTOT // N_CORES       # 512 token rows per core
NKT = D // 128              # 8 k-tiles over the model dim
NLT = LQ // 128             # 16 l-tiles per batch
VW = DK + 1                 # value tile width incl ones column
SCALE = 1.0 / float(np.sqrt(DK))
# (b, h) pairs whose col-softmax matrix is produced by DMA-transposing the
# row-softmax exp instead of a second scores+exp pass
OFFLOAD = {(0, 0), (1, 0)}

_CACHE = {}


def _build_program():
    import concourse.bacc as bacc
    import concourse.mybir as mybir
    from concourse import tile

    f32 = mybir.dt.float32
    f16 = mybir.dt.float16
    Exp = mybir.ActivationFunctionType.Exp
    add = mybir.AluOpType.add
    mult = mybir.AluOpType.mult

    nc = bacc.Bacc("TRN2", target_bir_lowering=False, debug=False,
                   num_devices=N_CORES)

    # ---- I/O ----
    query = nc.dram_tensor("query", [B, LQ, D], f32, kind="ExternalInput")
    context = nc.dram_tensor("context", [B, LC, D], f32, kind="ExternalInput")
    w0t = nc.dram_tensor("w0t", [D, DSL], f16, kind="ExternalInput")
    w1t = nc.dram_tensor("w1t", [D, DSL], f16, kind="ExternalInput")
    w2t = nc.dram_tensor("w2t", [D, DSL], f16, kind="ExternalInput")
    w3t = nc.dram_tensor("w3t", [D, DSL], f16, kind="ExternalInput")
    w4t = nc.dram_tensor("w4t", [D, D], f16, kind="ExternalInput")
    w5t = nc.dram_tensor("w5t", [D, D], f16, kind="ExternalInput")
    b0s = nc.dram_tensor("b0s", [DSL, 1], f32, kind="ExternalInput")
    b1s = nc.dram_tensor("b1s", [DSL, 1], f32, kind="ExternalInput")
    b2r = nc.dram_tensor("b2r", [128, DSL], f32, kind="ExternalInput")
    b3r = nc.dram_tensor("b3r", [128, DSL], f32, kind="ExternalInput")
    b4r = nc.dram_tensor("b4r", [128, D], f32, kind="ExternalInput")
    b5r = nc.dram_tensor("b5r", [128, D], f32, kind="ExternalInput")
    ident = nc.dram_tensor("ident", [128, 128], f16, kind="ExternalInput")
    out0c = nc.dram_tensor("out0c", [LSL, D], f32, kind="ExternalOutput")
    out1c = nc.dram_tensor("out1c", [LSL, D], f32, kind="ExternalOutput")

    with tile.TileContext(nc) as tc:
        with tc.tile_pool(name="dram", bufs=1, space="DRAM") as dram, \
             tc.tile_pool(name="const", bufs=1) as constp, \
             tc.tile_pool(name="psA", bufs=3, space="PSUM") as psA, \
             tc.tile_pool(name="psB", bufs=2, space="PSUM") as psB:

            # fp16 staging of the two activations (cast on SWDGE)
            stage_q = dram.tile([B, LQ, D], f16)
            stage_c = dram.tile([B, LC, D], f16)
            a2aq_in = dram.tile([N_CORES, DSL, LSL], f16)
            a2aq_out = dram.tile([N_CORES, DSL, LSL], f16)
            a2ac_in = dram.tile([N_CORES, DSL, LSL], f16)
            a2ac_out = dram.tile([N_CORES, DSL, LSL], f16)
            # DRAM bounce for the transpose-offloaded exp matrices
            etd = {bh: dram.tile([LC, LQ], f16, name=f"etd{bh[0]}_{bh[1]}")
                   for bh in OFFLOAD}

            for b in range(B):
                nc.gpsimd.dma_start(stage_q[b], query.ap()[b])
                nc.gpsimd.dma_start(stage_c[b], context.ap()[b])

            # constants / weights on the sync queue, emitted before the input
            # transposes (they fit in the window while the first cast runs)
            idt = constp.tile([128, 128], f16, name="idt")
            nc.sync.dma_start(idt[:], ident.ap())
            bias_qp = constp.tile([DSL, 1], f32, name="bias_qp")
            nc.sync.dma_start(bias_qp[:], b0s.ap())
            bias_cp = constp.tile([DSL, 1], f32, name="bias_cp")
            nc.sync.dma_start(bias_cp[:], b1s.ap())
            bias_qv = constp.tile([128, DSL], f32, name="bias_qv")
            nc.sync.dma_start(bias_qv[:], b2r.ap())
            bias_cv = constp.tile([128, DSL], f32, name="bias_cv")
            nc.sync.dma_start(bias_cv[:], b3r.ap())
            wq = [constp.tile([128, DSL], f16, name=f"wq{k}") for k in range(NKT)]
            wc = [constp.tile([128, DSL], f16, name=f"wc{k}") for k in range(NKT)]
            wqv = [constp.tile([128, DSL], f16, name=f"wqv{k}") for k in range(NKT)]
            wcv = [constp.tile([128, DSL], f16, name=f"wcv{k}") for k in range(NKT)]
            for k in range(NKT):
                sl = slice(128 * k, 128 * (k + 1))
                nc.sync.dma_start(wq[k][:], w0t.ap()[sl])
                nc.sync.dma_start(wc[k][:], w1t.ap()[sl])
                nc.sync.dma_start(wqv[k][:], w2t.ap()[sl])
                nc.sync.dma_start(wcv[k][:], w3t.ap()[sl])

            # ---- phase 1: transpose inputs + all projections ----
            with tc.tile_pool(name="proj", bufs=1) as projp:
                qTp = [projp.tile([128, LQ], f16, name=f"qTp{b}") for b in range(B)]
                cTp = [projp.tile([128, LC], f16, name=f"cTp{b}") for b in range(B)]
                # merged per-(batch, ltile) value tiles: cols [0:65] head 0
                # (ones at 64), [65:130] head 1 (ones at 129)
                qvv = [[projp.tile([128, 2 * VW], f16, name=f"qvv{b}_{lt}")
                        for lt in range(NLT)] for b in range(B)]
                cvv = [[projp.tile([128, 2 * VW], f16, name=f"cvv{b}_{lt}")
                        for lt in range(NLT)] for b in range(B)]

                with tc.tile_pool(name="inT", bufs=2 * NKT) as inp:
                    for b in range(B):
                        qT = [inp.tile([128, LQ], f16, tag="qT", name=f"qT{b}_{k}")
                              for k in range(NKT)]
                        cT = [inp.tile([128, LC], f16, tag="cT", name=f"cT{b}_{k}")
                              for k in range(NKT)]
                        for k in range(NKT):
                            dsl = slice(128 * k, 128 * (k + 1))
                            nc.sync.dma_start(qT[k][:], stage_q[b, :, dsl],
                                              transpose=True)
                            nc.sync.dma_start(cT[k][:], stage_c[b, :, dsl],
                                              transpose=True)
                        for (dst, w_, src, bias) in ((cTp, wc, cT, bias_cp),
                                                     (qTp, wq, qT, bias_qp)):
                            for ch in range(LQ // 512):
                                cs = slice(512 * ch, 512 * (ch + 1))
                                ps = psB.tile([128, 512], f32, tag="pss", name="ps")
                                for k in range(NKT):
                                    nc.tensor.matmul(ps[:], w_[k][:], src[k][:, cs],
                                                     start=(k == 0),
                                                     stop=(k == NKT - 1))
                                nc.vector.tensor_scalar(
                                    out=dst[b][:, cs], in0=ps[:],
                                    scalar1=bias[:, 0:1], scalar2=None, op0=add)
                        for (dst, w_, src, bias) in ((cvv, wcv, cT, bias_cv),
                                                     (qvv, wqv, qT, bias_qv)):
                            for lt in range(NLT):
                                ls = slice(128 * lt, 128 * (lt + 1))
                                ps = psB.tile([128, DSL], f32, tag="pss", name="ps")
                                for k in range(NKT):
                                    nc.tensor.matmul(ps[:], src[k][:, ls], w_[k][:],
                                                     start=(k == 0),
                                                     stop=(k == NKT - 1))
                                t = dst[b][lt]
                                for h in range(HPC):
                                    hs = slice(DK * h, DK * (h + 1))
                                    os = slice(VW * h, VW * h + DK)
                                    nc.vector.tensor_tensor(
                                        out=t[:, os], in0=ps[:, hs],
                                        in1=bias[:, hs], op=add)
                                    nc.vector.memset(
                                        t[:, VW * h + DK:VW * (h + 1)], 1.0)

                # ---- phase 2: attention ----
                with tc.tile_pool(name="att", bufs=1) as attp, \
                     tc.tile_pool(name="emat", bufs=26) as ematp:
                    rq = [[attp.tile([128, 128], f16, name=f"rq{b}_{m}")
                           for m in range(NLT)] for b in range(B)]
                    rc = [[attp.tile([128, 128], f16, name=f"rc{b}_{m}")
                           for m in range(NLT)] for b in range(B)]
                    rqt = [attp.tile([128, LQ], f16, name=f"rqt{b}") for b in range(B)]
                    rct = [attp.tile([128, LC], f16, name=f"rct{b}") for b in range(B)]

                    def scores_exp(b, h, lhsp, rhsp, dump=None):
                        """exp(S/sqrt(dk)) tiles: lhsT-tile k of the apply."""
                        hp = slice(64 * h, 64 * (h + 1))
                        et = []
                        for kt in range(NLT):
                            ks = slice(128 * kt, 128 * (kt + 1))
                            e = ematp.tile([128, 2048], f16, tag="et", name="e")
                            for ch in range(2):
                                cs = slice(1024 * ch, 1024 * (ch + 1))
                                sp = psA.tile([128, 1024], f32, tag="sps", name="sp")
                                nc.tensor.matmul(
                                    sp[:, 0:512], lhsp[hp, ks],
                                    rhsp[hp, 1024 * ch:1024 * ch + 512],
                                    start=True, stop=True)
                                nc.tensor.matmul(
                                    sp[:, 512:1024], lhsp[hp, ks],
                                    rhsp[hp, 1024 * ch + 512:1024 * (ch + 1)],
                                    start=True, stop=True)
                                nc.scalar.activation(e[:, cs], sp[:], Exp,
                                                     scale=SCALE)
                            if dump is not None:
                                nc.gpsimd.dma_start(dump[ks], e[:])
                            et.append(e)
                        return et

                    def apply_norm(et, vals, h, rdst):
                        hp = slice(64 * h, 64 * (h + 1))
                        vs = slice(VW * h, VW * (h + 1))
                        for m in range(NLT):
                            ms = slice(128 * m, 128 * (m + 1))
                            up = psB.tile([128, VW], f32, tag="pss", name="up")
                            for kt in range(NLT):
                                nc.tensor.matmul(up[:], et[kt][:, ms],
                                                 vals[kt][:, vs],
                                                 start=(kt == 0),
                                                 stop=(kt == NLT - 1))
                            rec = attp.tile([128, 1], f32, tag="rec", bufs=4,
                                            name="rec")
                            nc.vector.reciprocal(rec[:], up[:, DK:DK + 1])
                            nc.vector.tensor_scalar(
                                out=rdst[m][:, hp], in0=up[:, 0:DK],
                                scalar1=rec[:, 0:1], scalar2=None, op0=mult)

                    def shard_out(r, rt, b, a2a_in):
                        for m in range(NLT):
                            ms = slice(128 * m, 128 * (m + 1))
                            tp = psB.tile([128, 128], f16, tag="pss", name="tp")
                            nc.tensor.transpose(tp[:], r[b][m][:], idt[:])
                            nc.vector.tensor_copy(rt[b][:, ms], tp[:])
                        for j in range(4):
                            js = slice(512 * j, 512 * (j + 1))
                            nc.gpsimd.dma_start(a2a_in[4 * b + j], rt[b][:, js])

                    # U phase (row softmax -> q_res), dumping exp(S^T) to DRAM
                    # for the transpose-offloaded pairs
                    for b in range(B):
                        for h in range(HPC):
                            et = scores_exp(b, h, cTp[b], qTp[b],
                                            dump=(etd[(b, h)] if (b, h) in OFFLOAD
                                                  else None))
                            apply_norm(et, cvv[b], h, rq[b])
                        shard_out(rq, rqt, b, a2aq_in)

                    nc.gpsimd.collective_compute(
                        "AllToAll", mybir.AluOpType.bypass,
                        replica_groups=[list(range(N_CORES))],
                        ins=[a2aq_in.opt()], outs=[a2aq_out.opt()])

                    # V phase (col softmax -> c_res), non-offloaded head first
                    for b in range(B):
                        for h in sorted(range(HPC),
                                        key=lambda h_: (b, h_) in OFFLOAD):
                            if (b, h) in OFFLOAD:
                                et = []
                                for kt in range(NLT):
                                    ks = slice(128 * kt, 128 * (kt + 1))
                                    e = ematp.tile([128, 2048], f16, tag="et",
                                                   name="e")
                                    nc.sync.dma_start(e[:], etd[(b, h)][:, ks],
                                                      transpose=True)
                                    et.append(e)
                            else:
                                et = scores_exp(b, h, qTp[b], cTp[b])
                            apply_norm(et, qvv[b], h, rc[b])
                        shard_out(rc, rct, b, a2ac_in)

                    # out_q projection, hidden under the V phase: weights are
                    # streamed from DRAM through a small rotating pool
                    with tc.tile_pool(name="o0p", bufs=1) as o0p:
                        rqf = [o0p.tile([128, LSL], f16, name=f"rqf{k}")
                               for k in range(NKT)]
                        for k in range(NKT):
                            nc.gpsimd.dma_start(rqf[k][:], a2aq_out[k])
                        bias4 = o0p.tile([128, D], f32, name="bias4")
                        nc.gpsimd.dma_start(bias4[:], b4r.ap())
                        for mt in range(LSL // 128):
                            ms = slice(128 * mt, 128 * (mt + 1))
                            for ch in range(D // 512):
                                cs = slice(512 * ch, 512 * (ch + 1))
                                ps = psB.tile([128, 512], f32, tag="pss", name="ps")
                                for k in range(NKT):
                                    wk = o0p.tile([128, 512], f16, tag="w4s",
                                                  bufs=4, name="wk")
                                    nc.sync.dma_start(
                                        wk[:], w4t.ap()[128 * k:128 * (k + 1), cs])
                                    nc.tensor.matmul(ps[:], rqf[k][:, ms], wk[:],
                                                     start=(k == 0),
                                                     stop=(k == NKT - 1))
                                ev = o0p.tile([128, 512], f32, tag="oev", bufs=3,
                                              name="ev")
                                nc.vector.tensor_tensor(out=ev[:], in0=ps[:],
                                                        in1=bias4[:, cs], op=add)
                                nc.gpsimd.dma_start(out0c.ap()[ms, cs], ev[:])

                    nc.gpsimd.collective_compute(
                        "AllToAll", mybir.AluOpType.bypass,
                        replica_groups=[list(range(N_CORES))],
                        ins=[a2ac_in.opt()], outs=[a2ac_out.opt()])

            # ---- phase 3: out_c projection ----
            with tc.tile_pool(name="outp", bufs=1) as outp:
                w5 = [outp.tile([128, D], f16, name=f"w5_{k}") for k in range(NKT)]
                bias5 = outp.tile([128, D], f32, name="bias5")
                nc.sync.dma_start(bias5[:], b5r.ap())
                for k in range(NKT):
                    sl = slice(128 * k, 128 * (k + 1))
                    nc.sync.dma_start(w5[k][:], w5t.ap()[sl])
                rcf = [outp.tile([128, LSL], f16, name=f"rcf{k}") for k in range(NKT)]
                for k in range(NKT):
                    nc.sync.dma_start(rcf[k][:], a2ac_out[k])

                for mt in range(LSL // 128):
                    ms = slice(128 * mt, 128 * (mt + 1))
                    for ch in range(D // 512):
                        cs = slice(512 * ch, 512 * (ch + 1))
                        ps = psB.tile([128, 512], f32, tag="pss", name="ps")
                        for k in range(NKT):
                            nc.tensor.matmul(ps[:], rcf[k][:, ms], w5[k][:, cs],
                                             start=(k == 0), stop=(k == NKT - 1))
                        ev = outp.tile([128, 512], f32, tag="oev", bufs=3,
                                       name="ev")
                        nc.vector.tensor_tensor(out=ev[:], in0=ps[:],
                                                in1=bias5[:, cs], op=add)
                        nc.sync.dma_start(out1c.ap()[ms, cs], ev[:])

    nc.compile()
    return nc


def _prep_inputs(inputs):
    f16 = np.float16
    f32 = np.float32
    q = np.ascontiguousarray(np.asarray(inputs["query"], dtype=f32))
    c = np.ascontiguousarray(np.asarray(inputs["context"], dtype=f32))
    W = [np.asarray(inputs[f"W{i}"], dtype=f32) for i in range(6)]
    bias = [np.asarray(inputs[f"b{i}"], dtype=f32) for i in range(6)]
    ident = np.eye(128, dtype=f16)
    in_maps = []
    for k in range(N_CORES):
        dsl = slice(DSL * k, DSL * (k + 1))
        m = {
            "query": q,
            "context": c,
            "w0t": np.ascontiguousarray(W[0][dsl].T.astype(f16)),
            "w1t": np.ascontiguousarray(W[1][dsl].T.astype(f16)),
            "w2t": np.ascontiguousarray(W[2][dsl].T.astype(f16)),
            "w3t": np.ascontiguousarray(W[3][dsl].T.astype(f16)),
            "w4t": np.ascontiguousarray(W[4].T.astype(f16)),
            "w5t": np.ascontiguousarray(W[5].T.astype(f16)),
            "b0s": np.ascontiguousarray(bias[0][dsl].reshape(DSL, 1)),
            "b1s": np.ascontiguousarray(bias[1][dsl].reshape(DSL, 1)),
            "b2r": np.ascontiguousarray(np.tile(bias[2][dsl], (128, 1))),
            "b3r": np.ascontiguousarray(np.tile(bias[3][dsl], (128, 1))),
            "b4r": np.ascontiguousarray(np.tile(bias[4], (128, 1))),
            "b5r": np.ascontiguousarray(np.tile(bias[5], (128, 1))),
            "ident": ident,
        }
        in_maps.append(m)
    return in_maps


def _get_program():
    if "nc" not in _CACHE:
        _CACHE["nc"] = _build_program()
    return _CACHE["nc"]


def kernel(**inputs):
    from concourse.bass_utils import run_bass_kernel_spmd

    nc = _get_program()
    in_maps = _prep_inputs(inputs)
    res = run_bass_kernel_spmd(nc, in_maps, list(range(N_CORES)))
    out0 = np.concatenate([res.results[k]["out0c"] for k in range(N_CORES)], axis=0)
    out1 = np.concatenate([res.results[k]["out1c"] for k in range(N_CORES)], axis=0)
    return (out0.reshape(B, LQ, D).astype(np.float32),
            out1.reshape(B, LC, D).astype(np.float32))
